# revision 4
# baseline (speedup 1.0000x reference)
"""Self-contained trn2 Bass kernel: LSTM (B=64, T=512, I=128, H=1024,
forget_bias=1.0, tf gate order i,j,f,o) + per-timestep dense layer.

Strategy (8 NeuronCores): FULLY REPLICATED recurrence, sharded output.
Every core computes the complete LSTM recurrence locally (all 4096 gates,
full batch, transposed layout gates^T [4096 units, 64 batch] on the
TensorEngine); only the per-timestep dense layer is sharded (each core
produces its own 8-row batch slice of the output). There is NO inter-core
communication at all.

Why replication wins here (HW-measured on this axon/trn2 stack):
- Any per-step SWDGE remote-DMA broadcast costs ~11.6us wall-clock
  (66 ring descriptors at ~175ns each, independent of payload bytes), and
  any per-step GPSIMD Q7 engine op costs ~11us (launch+drain). A
  hidden-sharded recurrence therefore cannot go below ~11.6us/step.
- The fully replicated step is pure PE streaming: 256 Wh matmuls + 32 Wx
  matmuls of [128x128]x[128,64] = 18432 PE cycles/step ~ 9.6us/step,
  with the full-width cell (ScalarE 5 acts + VectorE 4 ops on [128,512])
  hiding underneath. That is the fastest point in the design space.

Numerics: identical to the sharded baseline (fp8-e4m3 weights for the
three sigmoid gates f,i,o; bf16 for the tanh candidate j; all weights
pre-scaled by 64 with activation scale=1/64; fp32 PSUM and cell state).

PSUM gate layout is gate-type-major: cols [f x8 tiles | j x8 | i x8 | o x8]
so one activation instruction covers each gate type ([128, 512]).

The per-core dense batch-slice is selected by ONE 8-way Switch around the
whole VectorE loop (per-step engine branches on DVE cost ~12us each,
HW-measured; a single branch is free; code is duplicated per core id).
"""

from contextlib import ExitStack

import numpy as np
import ml_dtypes

import concourse.bass as bass
import concourse.bacc as bacc
import concourse.mybir as mybir
from concourse.alu_op_type import AluOpType
from concourse.bass_utils import run_bass_kernel_spmd

F32 = mybir.dt.float32
BF16 = mybir.dt.bfloat16
FP8 = mybir.dt.float8e4
AF = mybir.ActivationFunctionType

N_CORES = 8
B = 64
H = 1024
NCH = 8                    # contraction chunks (1024 / 128)
NT = 8                     # 128-unit col tiles per gate type
GW = NT * B                # psum cols per gate type (512)

# gate types in PSUM order: f, j, i, o ; reference col order is i,j,f,o
TYPE_GATES = (2, 1, 0, 3)
# acts (asem): sf=5t+1, tj=5t+2, si=5t+3, so=5t+4, tc=5t+5
# vector (vsem): cf=3t+1, t1=3t+2, h=3t+3 ; csem: c -> t+2 ; histsem: copy(t) -> t+1


def _build(T, dense_blk, include_bias, include_dense_bias):
    assert T % dense_blk == 0 and dense_blk == 16
    n_blk = T // dense_blk
    MD = dense_blk * 8

    XT_CHUNKS = 8
    steps_per_chunk = T // XT_CHUNKS

    nc = bacc.Bacc(target_bir_lowering=False)

    xt_d = nc.declare_dram_parameter("XT", [128, T * B], BF16, isOutput=False)
    wx_d = nc.declare_dram_parameter("WX", [128, 4 * H], BF16, isOutput=False)
    wh8_d = nc.declare_dram_parameter("WH8", [H, 3 * H], FP8, isOutput=False)
    whb_d = nc.declare_dram_parameter("WHB", [H, H], BF16, isOutput=False)
    wd_d = nc.declare_dram_parameter("WD", [H, 128], BF16, isOutput=False)
    if include_bias:
        b_d = nc.declare_dram_parameter("B4096", [1, 4 * H], BF16, isOutput=False)
    if include_dense_bias:
        bd_d = nc.declare_dram_parameter("BD", [1, 128], BF16, isOutput=False)
    out_d = nc.declare_dram_parameter("OUT", [T * 8, 128], F32, isOutput=True)

    with ExitStack() as ctx:
        block = ctx.enter_context(nc.Block())
        sem = lambda n: ctx.enter_context(nc.semaphore(n))
        sb = lambda n, shape, dt: ctx.enter_context(nc.sbuf_tensor(n, shape, dt))

        wsem, initsem = sem("wsem"), sem("initsem")
        xsems = [sem(f"xsem{i}") for i in range(XT_CHUNKS)]
        gsem, asem, vsem, csem = sem("gsem"), sem("asem"), sem("vsem"), sem("csem")
        histsem, densesem, outcp, outdma, constsem = (
            sem("histsem"), sem("densesem"), sem("outcp"), sem("outdma"), sem("constsem"))

        xt_sb = sb("xt_sb", [128, T * B], BF16)
        wx_sb = sb("wx_sb", [128, 4 * H], BF16)
        wh8_sb = sb("wh8_sb", [128, NCH * 3 * H], FP8)
        whb_sb = sb("whb_sb", [128, NCH * H], BF16)
        wd_sb = sb("wd_sb", [128, NCH * 128], BF16)
        hT = sb("hT", [128, NCH * B], BF16)          # h^T, 8 chunks x 64 batch
        hist = sb("hist", [128, 2 * NCH * 128], BF16)
        c_sb = sb("c_sb", [128, GW], F32)
        sf_sb = sb("sf_sb", [128, GW], F32)
        tj_sb = sb("tj_sb", [128, GW], F32)
        si_sb = sb("si_sb", [128, GW], F32)
        so_sb = sb("so_sb", [128, GW], F32)
        tc_sb = sb("tc_sb", [128, GW], F32)
        cf_sb = sb("cf_sb", [128, GW], F32)
        t1_sb = sb("t1_sb", [128, GW], F32)
        ostage = sb("ostage", [128, 2 * 128], F32)
        gates_ps = ctx.enter_context(nc.psum_tensor("gates_ps", [128, 4 * GW], F32))
        dense_ps = ctx.enter_context(nc.psum_tensor("dense_ps", [128, 128], F32))
        if include_bias:
            ones_sb = sb("ones_sb", [1, B], BF16)
            b_sb = sb("b_sb", [1, 4 * H], BF16)
        if include_dense_bias:
            onesd_sb = sb("onesd_sb", [1, 128], BF16)
            bd_sb = sb("bd_sb", [1, 128], BF16)

        n_wdma = 4 + (1 if include_bias else 0) + (1 if include_dense_bias else 0)
        n_consts = (1 if include_bias else 0) + (1 if include_dense_bias else 0)

        def gtile(g, m):
            return gates_ps[:, (g * NT + m) * B : (g * NT + m) * B + B]

        def gtype(g):
            return gates_ps[:, g * GW : (g + 1) * GW]

        hT_r = hT[:, :].rearrange("p (c v) -> p c v", c=NCH)
        hist_r = hist[:, :].rearrange("p (q c w) -> p q c w", q=2, c=NCH)

        @block.sync
        def _(s: bass.BassEngine):
            s.dma_start(out=wx_sb[:, :], in_=wx_d[:, :]).then_inc(wsem, 16)
            s.dma_start(
                out=wh8_sb[:, :].rearrange("p (c g) -> p c g", c=NCH),
                in_=wh8_d[:, :].rearrange("(c p) g -> p c g", p=128),
            ).then_inc(wsem, 16)
            s.dma_start(
                out=whb_sb[:, :].rearrange("p (c g) -> p c g", c=NCH),
                in_=whb_d[:, :].rearrange("(c p) g -> p c g", p=128),
            ).then_inc(wsem, 16)
            s.dma_start(
                out=wd_sb[:, :].rearrange("p (c o) -> p c o", c=NCH),
                in_=wd_d[:, :].rearrange("(c p) o -> p c o", p=128),
            ).then_inc(wsem, 16)
            if include_bias:
                s.dma_start(out=b_sb[:, :], in_=b_d[:, :]).then_inc(wsem, 16)
            if include_dense_bias:
                s.dma_start(out=bd_sb[:, :], in_=bd_d[:, :]).then_inc(wsem, 16)
            w = steps_per_chunk * B
            for ci in range(XT_CHUNKS):
                s.dma_start(
                    out=xt_sb[:, ci * w : (ci + 1) * w],
                    in_=xt_d[:, ci * w : (ci + 1) * w],
                ).then_inc(xsems[ci], 16)
                s.wait_ge(xsems[ci], 16)
            for blk in range(n_blk):
                s.wait_ge(outcp, blk + 1)
                s.dma_start(
                    out=out_d[blk * MD : (blk + 1) * MD, :],
                    in_=ostage[:MD, (blk % 2) * 128 : (blk % 2) * 128 + 128],
                ).then_inc(outdma, 16)
                s.wait_ge(outdma, 16 * (blk + 1))

        @block.tensor
        def _(e: bass.BassTensorEngine):
            e.wait_ge(wsem, 16 * n_wdma)
            if n_consts:
                e.wait_ge(constsem, n_consts)
            e.wait_ge(initsem, 1)

            def dense_block(bi):
                e.wait_ge(histsem, dense_blk * (bi + 1))
                if bi >= 1:
                    e.wait_ge(outcp, bi)
                n_mm = NCH + (1 if include_dense_bias else 0)
                hoff = (bi % 2) * NCH * 128
                for c in range(NCH):
                    mm = e.matmul(
                        dense_ps[:MD, :],
                        lhsT=hist[:, hoff + c * 128 : hoff + (c + 1) * 128],
                        rhs=wd_sb[:, c * 128 : (c + 1) * 128],
                        start=(c == 0),
                        stop=(c == n_mm - 1),
                        skip_group_check=True,
                    )
                if include_dense_bias:
                    mm = e.matmul(
                        dense_ps[:MD, :],
                        lhsT=onesd_sb[0:1, :],
                        rhs=bd_sb[0:1, :],
                        start=False,
                        stop=True,
                        skip_group_check=True,
                    )
                mm.then_inc(densesem, 1)

            for t in range(T):
                if t % steps_per_chunk == 0:
                    e.wait_ge(xsems[t // steps_per_chunk], 16)
                # Wx matmuls (PSUM start); gate on previous step's act reads
                # act index per type (PSUM bank reader): f=1, j=2, i=3, o=5
                for g, act_n in zip(range(4), (1, 2, 3, 5)):
                    if t >= 1:
                        e.wait_ge(asem, 5 * (t - 1) + act_n)
                    for m in range(NT):
                        # start=True zeroes the whole 2KB PSUM bank (the
                        # pending-zero region is bank-granular), so only the
                        # first tile of each bank may start; siblings land on
                        # the pending-zero region and overwrite-as-first.
                        mm = e.matmul(
                            gtile(g, m),
                            lhsT=wx_sb[:, (g * NT + m) * 128 : (g * NT + m + 1) * 128],
                            rhs=xt_sb[:, t * B : (t + 1) * B],
                            start=(m == 0),
                            stop=False,
                            skip_group_check=True,
                        )
                        if include_bias:
                            mm = e.matmul(
                                gtile(g, m),
                                lhsT=b_sb[0:1, (g * NT + m) * 128 : (g * NT + m + 1) * 128],
                                rhs=ones_sb[0:1, :],
                                start=False,
                                stop=False,
                                skip_group_check=True,
                            )
                if t >= 20 and (t - 20) % dense_blk == 0:
                    dense_block((t - 20) // dense_blk)
                # Wh matmuls; h_{t-1} ready once vector h op of t-1 is done
                if t >= 1:
                    e.wait_ge(vsem, 3 * t)
                for g in range(4):
                    for m in range(NT):
                        for c in range(NCH):
                            if g == 1:
                                lhsT = whb_sb[
                                    :, (c * NT + m) * 128 : (c * NT + m + 1) * 128
                                ]
                            else:
                                sub = (0, None, 1, 2)[g]
                                lhsT = wh8_sb[
                                    :,
                                    (c * 3 * NT + sub * NT + m) * 128 : (c * 3 * NT + sub * NT + m + 1) * 128,
                                ]
                            mm = e.matmul(
                                gtile(g, m),
                                lhsT=lhsT,
                                rhs=hT_r[:, c, 0:B],
                                start=False,
                                stop=(c == NCH - 1),
                                skip_group_check=True,
                            )
                    if m == NT - 1:
                        mm.then_inc(gsem, 1)
            b0 = (T - 20) // dense_blk + 1
            for bi in range(b0, n_blk):
                dense_block(bi)

        @block.scalar
        def _(a: bass.BassScalarEngine):
            for t in range(T):
                # sf = sigmoid(f/64 + 1)
                a.wait_ge(gsem, 4 * t + 1)
                if t >= 1:
                    a.wait_ge(vsem, 3 * (t - 1) + 2)
                a.activation(sf_sb[:, :], gtype(0), AF.Sigmoid, bias=1.0,
                             scale=1.0 / 64.0).then_inc(asem, 1)
                a.wait_ge(gsem, 4 * t + 2)
                a.activation(tj_sb[:, :], gtype(1), AF.Tanh,
                             scale=1.0 / 64.0).then_inc(asem, 1)
                a.wait_ge(gsem, 4 * t + 3)
                a.activation(si_sb[:, :], gtype(2), AF.Sigmoid,
                             scale=1.0 / 64.0).then_inc(asem, 1)
                # tc BEFORE so: tanh(c_t) only needs f,j,i gates, so it runs
                # while PE still streams the o tiles; the post-matmul serial
                # tail is then just sigmoid(o) + the h mult.
                a.wait_ge(csem, t + 2)
                if t >= 1:
                    a.wait_ge(vsem, 3 * t)
                a.activation(tc_sb[:, :], c_sb[:, :], AF.Tanh).then_inc(asem, 1)
                a.wait_ge(gsem, 4 * t + 4)
                a.activation(so_sb[:, :], gtype(3), AF.Sigmoid,
                             scale=1.0 / 64.0).then_inc(asem, 1)
                # ostage copy for dense block b at t = 16b+22
                if t >= 22 and (t - 22) % dense_blk == 0:
                    bi = (t - 22) // dense_blk
                    a.wait_ge(densesem, bi + 1)
                    if bi >= 2:
                        a.wait_ge(outdma, 16 * (bi - 1))
                    a.copy(
                        ostage[:MD, (bi % 2) * 128 : (bi % 2) * 128 + 128],
                        dense_ps[:MD, :],
                    ).then_inc(outcp, 1)
            b0 = (T - 22) // dense_blk + 1
            for bi in range(b0, n_blk):
                a.wait_ge(densesem, bi + 1)
                if bi >= 2:
                    a.wait_ge(outdma, 16 * (bi - 1))
                a.copy(
                    ostage[:MD, (bi % 2) * 128 : (bi % 2) * 128 + 128],
                    dense_ps[:MD, :],
                ).then_inc(outcp, 1)

        @block.vector
        def _(v: bass.BassVectorEngine):
            myg = v.partition_id()

            if include_bias:
                v.memset(ones_sb[:, :], 1.0).then_inc(constsem, 1)
            if include_dense_bias:
                v.memset(onesd_sb[:, :], 1.0).then_inc(constsem, 1)
            v.memset(c_sb[:, :], 0.0).then_inc(csem, 1)
            v.memset(hT[:, :], 0.0).then_inc(initsem, 1)

            def hist_copy(vv, s, k):
                # pack own batch-slice of step s from hT into hist; runs right
                # after the h write of step s (same-engine, vsem-ordered).
                if s % dense_blk == 0 and s // dense_blk >= 2:
                    vv.wait_ge(densesem, s // dense_blk - 1)
                tl = (s % dense_blk) * 8
                dst = hist_r[:, (s // dense_blk) % 2, :, tl : tl + 8]
                src = hT_r[:, :, k * 8 : k * 8 + 8]
                vv.tensor_copy(dst, src).then_inc(histsem, 1)

            # ONE 8-way Switch around the whole loop (per-step DVE branches
            # cost ~12us each, HW-measured; a single branch is free).
            for myk in v.Switch(myg, N_CORES):
                for t in range(T):
                    # cf = sf * c_{t-1}
                    v.wait_ge(asem, 5 * t + 1)
                    v.wait_ge(csem, t + 1)
                    v.tensor_tensor(
                        cf_sb[:, :], sf_sb[:, :], c_sb[:, :], AluOpType.mult
                    ).then_inc(vsem, 1)
                    # t1 = si * tj
                    v.wait_ge(asem, 5 * t + 3)
                    v.tensor_tensor(
                        t1_sb[:, :], si_sb[:, :], tj_sb[:, :], AluOpType.mult
                    ).then_inc(vsem, 1)
                    # c_t = cf + t1 (tc(t-1) done at asem 5t; cf/t1 via vsem)
                    v.wait_ge(asem, 5 * t)
                    v.wait_ge(vsem, 3 * t + 2)
                    v.tensor_tensor(
                        c_sb[:, :], cf_sb[:, :], t1_sb[:, :], AluOpType.add
                    ).then_inc(csem, 1)
                    # h_t = so * tc -> hT (bf16); hist copy of t-1 must be done
                    v.wait_ge(asem, 5 * t + 5)
                    if t >= 1:
                        v.wait_ge(histsem, t)
                    v.tensor_tensor(
                        hT[:, :], so_sb[:, :], tc_sb[:, :], AluOpType.mult
                    ).then_inc(vsem, 1)
                    # pack own dense slice of h_t (reads hT after the h write)
                    v.wait_ge(vsem, 3 * t + 3)
                    hist_copy(v, t, myk)

    nc.finalize()
    return nc


_BUILD_CACHE = {}


def build(T, dense_blk, include_bias, include_dense_bias):
    return _build(T, dense_blk, include_bias, include_dense_bias)


def prep_inputs(X, Wx, Wh, b, Wd, bd):
    X = np.asarray(X, dtype=np.float32)
    Wx = np.asarray(Wx, dtype=np.float32)
    Wh = np.asarray(Wh, dtype=np.float32)
    b = np.asarray(b, dtype=np.float32)
    Wd = np.asarray(Wd, dtype=np.float32)
    bd = np.asarray(bd, dtype=np.float32)
    Bsz, T, _ = X.shape
    include_bias = bool(np.any(b))
    include_dense_bias = bool(np.any(bd))
    bf = ml_dtypes.bfloat16
    f8 = ml_dtypes.float8_e4m3fn
    XT = np.ascontiguousarray(np.transpose(X, (2, 1, 0))).reshape(128, T * Bsz)
    # gate-type-major col order (f, j, i, o), 8 x 128-unit tiles per type
    cols = np.concatenate([np.arange(g * H, (g + 1) * H) for g in TYPE_GATES])
    # fp8 types f, i, o ; bf16 type j
    cols8 = np.concatenate(
        [np.arange(g * H, (g + 1) * H) for g in (TYPE_GATES[0], TYPE_GATES[2], TYPE_GATES[3])]
    )
    colsb = np.arange(TYPE_GATES[1] * H, (TYPE_GATES[1] + 1) * H)
    m = {
        "XT": XT.astype(bf),
        "WX": np.ascontiguousarray(64.0 * Wx[:, cols]).astype(bf),
        "WH8": np.ascontiguousarray(64.0 * Wh[:, cols8]).astype(f8),
        "WHB": np.ascontiguousarray(64.0 * Wh[:, colsb]).astype(bf),
        "WD": Wd.astype(bf),
    }
    if include_bias:
        m["B4096"] = np.ascontiguousarray(64.0 * b[cols])[None, :].astype(bf)
    if include_dense_bias:
        m["BD"] = np.ascontiguousarray(bd)[None, :].astype(bf)
    return [dict(m) for _ in range(N_CORES)]


def assemble_output(results_list, T):
    outs = []
    for k in range(N_CORES):
        o = np.asarray(results_list[k]["OUT"]).reshape(T, 8, 128).transpose(1, 0, 2)
        outs.append(o)
    return np.concatenate(outs, axis=0).astype(np.float32)


def kernel(X, Wx, Wh, b, Wd, bd):
    X = np.asarray(X, dtype=np.float32)
    Bsz, T, _ = X.shape
    assert Bsz == B
    dense_blk = 16
    include_bias = bool(np.any(np.asarray(b)))
    include_dense_bias = bool(np.any(np.asarray(bd)))

    key = (T, dense_blk, include_bias, include_dense_bias)
    if key not in _BUILD_CACHE:
        _BUILD_CACHE[key] = _build(T, dense_blk, include_bias, include_dense_bias)
    nc = _BUILD_CACHE[key]

    in_maps = prep_inputs(X, Wx, Wh, b, Wd, bd)
    res = None
    for attempt in range(3):
        try:
            res = run_bass_kernel_spmd(nc, in_maps, core_ids=list(range(N_CORES)))
            break
        except Exception:
            if attempt == 2:
                raise
    return assemble_output([res.results[k] for k in range(N_CORES)], T=T)


# revision 5
# speedup vs baseline: 1.1379x; 1.1379x over previous
"""Self-contained trn2 Bass kernel: LSTM (B=64, T=512, I=128, H=1024,
forget_bias=1.0, tf gate order i,j,f,o) + per-timestep dense layer.

Strategy (8 NeuronCores): FULLY REPLICATED recurrence, sharded output.
Every core computes the complete LSTM recurrence locally (all 4096 gates,
full batch, transposed layout gates^T [4096 units, 64 batch] on the
TensorEngine); only the per-timestep dense layer is sharded (each core
produces its own 8-row batch slice of the output). There is NO inter-core
communication at all.

Why replication wins here (HW-measured on this axon/trn2 stack):
- Any per-step SWDGE remote-DMA broadcast costs ~11.6us wall-clock
  (66 ring descriptors at ~175ns each, independent of payload bytes), and
  any per-step GPSIMD Q7 engine op costs ~11us (launch+drain). A
  hidden-sharded recurrence therefore cannot go below ~11.6us/step.
- The fully replicated step is pure PE streaming: 256 Wh matmuls + 32 Wx
  matmuls of [128x128]x[128,64] = 18432 PE cycles/step ~ 9.6us/step,
  with the full-width cell (ScalarE 5 acts + VectorE 4 ops on [128,512])
  hiding underneath. That is the fastest point in the design space.

Numerics: identical to the sharded baseline (fp8-e4m3 weights for the
three sigmoid gates f,i,o; bf16 for the tanh candidate j; all weights
pre-scaled by 64 with activation scale=1/64; fp32 PSUM and cell state).

PSUM gate layout is gate-type-major: cols [f x8 tiles | j x8 | i x8 | o x8]
so one activation instruction covers each gate type ([128, 512]).

The per-core dense batch-slice is selected by ONE 8-way Switch around the
whole VectorE loop (per-step engine branches on DVE cost ~12us each,
HW-measured; a single branch is free; code is duplicated per core id).
"""

from contextlib import ExitStack

import numpy as np
import ml_dtypes

import concourse.bass as bass
import concourse.bacc as bacc
import concourse.mybir as mybir
from concourse.alu_op_type import AluOpType
from concourse.bass_utils import run_bass_kernel_spmd

F32 = mybir.dt.float32
BF16 = mybir.dt.bfloat16
FP8 = mybir.dt.float8e4
AF = mybir.ActivationFunctionType

N_CORES = 8
B = 64
H = 1024
NCH = 8                    # contraction chunks (1024 / 128)
NT = 8                     # 128-unit col tiles per gate type
GW = NT * B                # psum cols per gate type (512)

# gate types in PSUM order: f, j, i, o ; reference col order is i,j,f,o
TYPE_GATES = (2, 1, 0, 3)
# acts (asem): sf=5t+1, tj=5t+2, si=5t+3, so=5t+4, tc=5t+5
# vector (vsem): cf=3t+1, t1=3t+2, h=3t+3 ; csem: c -> t+2 ; histsem: copy(t) -> t+1


def _build(T, dense_blk, include_bias, include_dense_bias):
    assert T % dense_blk == 0 and dense_blk == 16
    n_blk = T // dense_blk
    MD = dense_blk * 8

    XT_CHUNKS = 8
    steps_per_chunk = T // XT_CHUNKS

    nc = bacc.Bacc(target_bir_lowering=False)

    xt_d = nc.declare_dram_parameter("XT", [128, T * B], BF16, isOutput=False)
    wx_d = nc.declare_dram_parameter("WX", [128, 4 * H], BF16, isOutput=False)
    wh8_d = nc.declare_dram_parameter("WH8", [H, 3 * H], FP8, isOutput=False)
    whb_d = nc.declare_dram_parameter("WHB", [H, H], BF16, isOutput=False)
    wd_d = nc.declare_dram_parameter("WD", [H, 128], BF16, isOutput=False)
    if include_bias:
        b_d = nc.declare_dram_parameter("B4096", [1, 4 * H], BF16, isOutput=False)
    if include_dense_bias:
        bd_d = nc.declare_dram_parameter("BD", [1, 128], BF16, isOutput=False)
    out_d = nc.declare_dram_parameter("OUT", [T * 8, 128], F32, isOutput=True)

    with ExitStack() as ctx:
        block = ctx.enter_context(nc.Block())
        sem = lambda n: ctx.enter_context(nc.semaphore(n))
        sb = lambda n, shape, dt: ctx.enter_context(nc.sbuf_tensor(n, shape, dt))

        wsem, initsem = sem("wsem"), sem("initsem")
        xsems = [sem(f"xsem{i}") for i in range(XT_CHUNKS)]
        gsem, asem, vsem, csem = sem("gsem"), sem("asem"), sem("vsem"), sem("csem")
        histsem, densesem, outcp, outdma, constsem = (
            sem("histsem"), sem("densesem"), sem("outcp"), sem("outdma"), sem("constsem"))

        xt_sb = sb("xt_sb", [128, T * B], BF16)
        wx_sb = sb("wx_sb", [128, 4 * H], BF16)
        wh8_sb = sb("wh8_sb", [128, NCH * 3 * H], FP8)
        whb_sb = sb("whb_sb", [128, NCH * H], BF16)
        wd_sb = sb("wd_sb", [128, NCH * 128], BF16)
        hT = sb("hT", [128, NCH * B], BF16)          # h^T, 8 chunks x 64 batch
        hist = sb("hist", [128, 2 * NCH * 128], BF16)
        c_sb = sb("c_sb", [128, GW], F32)
        sf_sb = sb("sf_sb", [128, GW], F32)
        tj_sb = sb("tj_sb", [128, GW], F32)
        si_sb = sb("si_sb", [128, GW], F32)
        so_sb = sb("so_sb", [128, GW], F32)
        tc_sb = sb("tc_sb", [128, GW], F32)
        cf_sb = sb("cf_sb", [128, GW], F32)
        t1_sb = sb("t1_sb", [128, GW], F32)
        ostage = sb("ostage", [128, 2 * 128], F32)
        gates_ps = ctx.enter_context(nc.psum_tensor("gates_ps", [128, 4 * GW], F32))
        dense_ps = ctx.enter_context(nc.psum_tensor("dense_ps", [128, 128], F32))
        if include_bias:
            ones_sb = sb("ones_sb", [1, B], BF16)
            b_sb = sb("b_sb", [1, 4 * H], BF16)
        if include_dense_bias:
            onesd_sb = sb("onesd_sb", [1, 128], BF16)
            bd_sb = sb("bd_sb", [1, 128], BF16)

        n_wdma = 4 + (1 if include_bias else 0) + (1 if include_dense_bias else 0)
        n_consts = (1 if include_bias else 0) + (1 if include_dense_bias else 0)

        def gtile(g, m):
            return gates_ps[:, (g * NT + m) * B : (g * NT + m) * B + B]

        def gtype(g):
            return gates_ps[:, g * GW : (g + 1) * GW]

        hT_r = hT[:, :].rearrange("p (c v) -> p c v", c=NCH)
        hist_r = hist[:, :].rearrange("p (q c w) -> p q c w", q=2, c=NCH)

        @block.sync
        def _(s: bass.BassEngine):
            s.dma_start(out=wx_sb[:, :], in_=wx_d[:, :]).then_inc(wsem, 16)
            s.dma_start(
                out=wh8_sb[:, :].rearrange("p (c g) -> p c g", c=NCH),
                in_=wh8_d[:, :].rearrange("(c p) g -> p c g", p=128),
            ).then_inc(wsem, 16)
            s.dma_start(
                out=whb_sb[:, :].rearrange("p (c g) -> p c g", c=NCH),
                in_=whb_d[:, :].rearrange("(c p) g -> p c g", p=128),
            ).then_inc(wsem, 16)
            s.dma_start(
                out=wd_sb[:, :].rearrange("p (c o) -> p c o", c=NCH),
                in_=wd_d[:, :].rearrange("(c p) o -> p c o", p=128),
            ).then_inc(wsem, 16)
            if include_bias:
                s.dma_start(out=b_sb[:, :], in_=b_d[:, :]).then_inc(wsem, 16)
            if include_dense_bias:
                s.dma_start(out=bd_sb[:, :], in_=bd_d[:, :]).then_inc(wsem, 16)
            w = steps_per_chunk * B
            for ci in range(XT_CHUNKS):
                s.dma_start(
                    out=xt_sb[:, ci * w : (ci + 1) * w],
                    in_=xt_d[:, ci * w : (ci + 1) * w],
                ).then_inc(xsems[ci], 16)
                s.wait_ge(xsems[ci], 16)
            for blk in range(n_blk):
                s.wait_ge(outcp, blk + 1)
                s.dma_start(
                    out=out_d[blk * MD : (blk + 1) * MD, :],
                    in_=ostage[:MD, (blk % 2) * 128 : (blk % 2) * 128 + 128],
                ).then_inc(outdma, 16)
                s.wait_ge(outdma, 16 * (blk + 1))

        @block.tensor
        def _(e: bass.BassTensorEngine):
            e.wait_ge(wsem, 16 * n_wdma)
            if n_consts:
                e.wait_ge(constsem, n_consts)
            e.wait_ge(initsem, 1)

            def dense_block(bi):
                e.wait_ge(histsem, dense_blk * (bi + 1))
                if bi >= 1:
                    e.wait_ge(outcp, bi)
                n_mm = NCH + (1 if include_dense_bias else 0)
                hoff = (bi % 2) * NCH * 128
                for c in range(NCH):
                    mm = e.matmul(
                        dense_ps[:MD, :],
                        lhsT=hist[:, hoff + c * 128 : hoff + (c + 1) * 128],
                        rhs=wd_sb[:, c * 128 : (c + 1) * 128],
                        start=(c == 0),
                        stop=(c == n_mm - 1),
                        skip_group_check=True,
                    )
                if include_dense_bias:
                    mm = e.matmul(
                        dense_ps[:MD, :],
                        lhsT=onesd_sb[0:1, :],
                        rhs=bd_sb[0:1, :],
                        start=False,
                        stop=True,
                        skip_group_check=True,
                    )
                mm.then_inc(densesem, 1)

            for t in range(T):
                if t % steps_per_chunk == 0:
                    e.wait_ge(xsems[t // steps_per_chunk], 16)
                # Wx matmuls (PSUM start); gate on previous step's act reads
                for g in range(4):
                    if t >= 1:
                        e.wait_ge(asem, 5 * (t - 1) + g + 1)
                    for m in range(NT):
                        # start=True zeroes the whole 2KB PSUM bank (the
                        # pending-zero region is bank-granular), so only the
                        # first tile of each bank may start; siblings land on
                        # the pending-zero region and overwrite-as-first.
                        mm = e.matmul(
                            gtile(g, m),
                            lhsT=wx_sb[:, (g * NT + m) * 128 : (g * NT + m + 1) * 128],
                            rhs=xt_sb[:, t * B : (t + 1) * B],
                            start=(m == 0),
                            stop=False,
                            skip_group_check=True,
                        )
                        if include_bias:
                            mm = e.matmul(
                                gtile(g, m),
                                lhsT=b_sb[0:1, (g * NT + m) * 128 : (g * NT + m + 1) * 128],
                                rhs=ones_sb[0:1, :],
                                start=False,
                                stop=False,
                                skip_group_check=True,
                            )
                if t >= 20 and (t - 20) % dense_blk == 0:
                    dense_block((t - 20) // dense_blk)
                # Wh matmuls; h_{t-1} ready once vector h op of t-1 is done
                if t >= 1:
                    e.wait_ge(vsem, 3 * t)
                for g in range(4):
                    for m in range(NT):
                        for c in range(NCH):
                            if g == 1:
                                lhsT = whb_sb[
                                    :, (c * NT + m) * 128 : (c * NT + m + 1) * 128
                                ]
                            else:
                                sub = (0, None, 1, 2)[g]
                                lhsT = wh8_sb[
                                    :,
                                    (c * 3 * NT + sub * NT + m) * 128 : (c * 3 * NT + sub * NT + m + 1) * 128,
                                ]
                            mm = e.matmul(
                                gtile(g, m),
                                lhsT=lhsT,
                                rhs=hT_r[:, c, 0:B],
                                start=False,
                                stop=(c == NCH - 1),
                                skip_group_check=True,
                            )
                    if m == NT - 1:
                        mm.then_inc(gsem, 1)
            b0 = (T - 20) // dense_blk + 1
            for bi in range(b0, n_blk):
                dense_block(bi)

        @block.scalar
        def _(a: bass.BassScalarEngine):
            for t in range(T):
                # sf = sigmoid(f/64 + 1)
                a.wait_ge(gsem, 4 * t + 1)
                if t >= 1:
                    a.wait_ge(vsem, 3 * (t - 1) + 2)
                a.activation(sf_sb[:, :], gtype(0), AF.Sigmoid, bias=1.0,
                             scale=1.0 / 64.0).then_inc(asem, 1)
                a.wait_ge(gsem, 4 * t + 2)
                a.activation(tj_sb[:, :], gtype(1), AF.Tanh,
                             scale=1.0 / 64.0).then_inc(asem, 1)
                a.wait_ge(gsem, 4 * t + 3)
                a.activation(si_sb[:, :], gtype(2), AF.Sigmoid,
                             scale=1.0 / 64.0).then_inc(asem, 1)
                a.wait_ge(gsem, 4 * t + 4)
                if t >= 1:
                    a.wait_ge(vsem, 3 * t)
                a.activation(so_sb[:, :], gtype(3), AF.Sigmoid,
                             scale=1.0 / 64.0).then_inc(asem, 1)
                a.wait_ge(csem, t + 2)
                a.activation(tc_sb[:, :], c_sb[:, :], AF.Tanh).then_inc(asem, 1)
                # ostage copy for dense block b at t = 16b+22
                if t >= 22 and (t - 22) % dense_blk == 0:
                    bi = (t - 22) // dense_blk
                    a.wait_ge(densesem, bi + 1)
                    if bi >= 2:
                        a.wait_ge(outdma, 16 * (bi - 1))
                    a.copy(
                        ostage[:MD, (bi % 2) * 128 : (bi % 2) * 128 + 128],
                        dense_ps[:MD, :],
                    ).then_inc(outcp, 1)
            b0 = (T - 22) // dense_blk + 1
            for bi in range(b0, n_blk):
                a.wait_ge(densesem, bi + 1)
                if bi >= 2:
                    a.wait_ge(outdma, 16 * (bi - 1))
                a.copy(
                    ostage[:MD, (bi % 2) * 128 : (bi % 2) * 128 + 128],
                    dense_ps[:MD, :],
                ).then_inc(outcp, 1)

        @block.vector
        def _(v: bass.BassVectorEngine):
            myg = v.partition_id()

            if include_bias:
                v.memset(ones_sb[:, :], 1.0).then_inc(constsem, 1)
            if include_dense_bias:
                v.memset(onesd_sb[:, :], 1.0).then_inc(constsem, 1)
            v.memset(c_sb[:, :], 0.0).then_inc(csem, 1)
            v.memset(hT[:, :], 0.0).then_inc(initsem, 1)

            def hist_copy(vv, s, k):
                # pack own batch-slice of step s from hT into hist; runs right
                # after the h write of step s (same-engine, vsem-ordered).
                if s % dense_blk == 0 and s // dense_blk >= 2:
                    vv.wait_ge(densesem, s // dense_blk - 1)
                tl = (s % dense_blk) * 8
                dst = hist_r[:, (s // dense_blk) % 2, :, tl : tl + 8]
                src = hT_r[:, :, k * 8 : k * 8 + 8]
                vv.tensor_copy(dst, src).then_inc(histsem, 1)

            # ONE 8-way Switch around the whole loop (per-step DVE branches
            # cost ~12us each, HW-measured; a single branch is free).
            for myk in v.Switch(myg, N_CORES):
                for t in range(T):
                    # cf = sf * c_{t-1}
                    v.wait_ge(asem, 5 * t + 1)
                    v.wait_ge(csem, t + 1)
                    v.tensor_tensor(
                        cf_sb[:, :], sf_sb[:, :], c_sb[:, :], AluOpType.mult
                    ).then_inc(vsem, 1)
                    # t1 = si * tj
                    v.wait_ge(asem, 5 * t + 3)
                    v.tensor_tensor(
                        t1_sb[:, :], si_sb[:, :], tj_sb[:, :], AluOpType.mult
                    ).then_inc(vsem, 1)
                    # c_t = cf + t1 (tc(t-1) done at asem 5t; cf/t1 via vsem)
                    v.wait_ge(asem, 5 * t)
                    v.wait_ge(vsem, 3 * t + 2)
                    v.tensor_tensor(
                        c_sb[:, :], cf_sb[:, :], t1_sb[:, :], AluOpType.add
                    ).then_inc(csem, 1)
                    # h_t = so * tc -> hT (bf16); hist copy of t-1 must be done
                    v.wait_ge(asem, 5 * t + 5)
                    if t >= 1:
                        v.wait_ge(histsem, t)
                    v.tensor_tensor(
                        hT[:, :], so_sb[:, :], tc_sb[:, :], AluOpType.mult
                    ).then_inc(vsem, 1)
                    # pack own dense slice of h_t (reads hT after the h write)
                    v.wait_ge(vsem, 3 * t + 3)
                    hist_copy(v, t, myk)

    nc.finalize()
    return nc


_BUILD_CACHE = {}


def build(T, dense_blk, include_bias, include_dense_bias):
    return _build(T, dense_blk, include_bias, include_dense_bias)


def prep_inputs(X, Wx, Wh, b, Wd, bd):
    X = np.asarray(X, dtype=np.float32)
    Wx = np.asarray(Wx, dtype=np.float32)
    Wh = np.asarray(Wh, dtype=np.float32)
    b = np.asarray(b, dtype=np.float32)
    Wd = np.asarray(Wd, dtype=np.float32)
    bd = np.asarray(bd, dtype=np.float32)
    Bsz, T, _ = X.shape
    include_bias = bool(np.any(b))
    include_dense_bias = bool(np.any(bd))
    bf = ml_dtypes.bfloat16
    f8 = ml_dtypes.float8_e4m3fn
    XT = np.ascontiguousarray(np.transpose(X, (2, 1, 0))).reshape(128, T * Bsz)
    # gate-type-major col order (f, j, i, o), 8 x 128-unit tiles per type
    cols = np.concatenate([np.arange(g * H, (g + 1) * H) for g in TYPE_GATES])
    # fp8 types f, i, o ; bf16 type j
    cols8 = np.concatenate(
        [np.arange(g * H, (g + 1) * H) for g in (TYPE_GATES[0], TYPE_GATES[2], TYPE_GATES[3])]
    )
    colsb = np.arange(TYPE_GATES[1] * H, (TYPE_GATES[1] + 1) * H)
    m = {
        "XT": XT.astype(bf),
        "WX": np.ascontiguousarray(64.0 * Wx[:, cols]).astype(bf),
        "WH8": np.ascontiguousarray(64.0 * Wh[:, cols8]).astype(f8),
        "WHB": np.ascontiguousarray(64.0 * Wh[:, colsb]).astype(bf),
        "WD": Wd.astype(bf),
    }
    if include_bias:
        m["B4096"] = np.ascontiguousarray(64.0 * b[cols])[None, :].astype(bf)
    if include_dense_bias:
        m["BD"] = np.ascontiguousarray(bd)[None, :].astype(bf)
    return [dict(m) for _ in range(N_CORES)]


def assemble_output(results_list, T):
    outs = []
    for k in range(N_CORES):
        o = np.asarray(results_list[k]["OUT"]).reshape(T, 8, 128).transpose(1, 0, 2)
        outs.append(o)
    return np.concatenate(outs, axis=0).astype(np.float32)


def kernel(X, Wx, Wh, b, Wd, bd):
    X = np.asarray(X, dtype=np.float32)
    Bsz, T, _ = X.shape
    assert Bsz == B
    dense_blk = 16
    include_bias = bool(np.any(np.asarray(b)))
    include_dense_bias = bool(np.any(np.asarray(bd)))

    key = (T, dense_blk, include_bias, include_dense_bias)
    if key not in _BUILD_CACHE:
        _BUILD_CACHE[key] = _build(T, dense_blk, include_bias, include_dense_bias)
    nc = _BUILD_CACHE[key]

    in_maps = prep_inputs(X, Wx, Wh, b, Wd, bd)
    res = None
    for attempt in range(3):
        try:
            res = run_bass_kernel_spmd(nc, in_maps, core_ids=list(range(N_CORES)))
            break
        except Exception:
            if attempt == 2:
                raise
    return assemble_output([res.results[k] for k in range(N_CORES)], T=T)
